# revision 15
# baseline (speedup 1.0000x reference)
import sys

sys.path.insert(0, "/opt/trn_rl_repo")

import numpy as np
import ml_dtypes
import concourse.bass as bass
import concourse.bacc as bacc
import concourse.mybir as mybir
from concourse.tile import TileContext
from concourse.bass_utils import run_bass_kernel_spmd

B, T, D = 2048, 200, 64
H1, H2 = 128, 64
NCORES = 8
BLOC = B // NCORES  # 256 batches per core
BB = 128            # batches per block
NBLK = BLOC // BB   # 2

F32 = mybir.dt.float32
BF16 = mybir.dt.bfloat16
I8 = mybir.dt.int8
KSCALE = 32.0  # k int8 quant scale; 1/KSCALE is folded into Wk/Wd
NPBF = ml_dtypes.bfloat16
AF = mybir.ActivationFunctionType
ALU = mybir.AluOpType
AX = mybir.AxisListType

_cached = {}


# ---------------------------------------------------------------------------
# Cached-jit execution path: run_bass_via_pjrt builds a fresh jax.jit (and
# re-traces/stages) on every call, costing ~0.3s. Cache the sharded jit per
# Bass module and reuse it across calls; fall back to the stock path on any
# error. run_bass_kernel_spmd dispatches through this module attribute.
from concourse import bass2jax as _b2j

_SPMD_CACHE = {}
_ORIG_RUN_VIA_PJRT = _b2j.run_bass_via_pjrt


def _build_spmd_entry(nc, n_cores):
    import jax
    from jax.sharding import Mesh, PartitionSpec
    from jax.experimental.shard_map import shard_map

    _b2j.install_neuronx_cc_hook()
    partition_name = nc.partition_id_tensor.name if nc.partition_id_tensor else None
    in_names, out_names, out_avals, zero_outs = [], [], [], []
    for alloc in nc.m.functions[0].allocations:
        if not isinstance(alloc, mybir.MemoryLocationSet):
            continue
        name = alloc.memorylocations[0].name
        if alloc.kind == "ExternalInput":
            if name != partition_name:
                in_names.append(name)
        elif alloc.kind == "ExternalOutput":
            shape = tuple(alloc.tensor_shape)
            dtype = mybir.dt.np(alloc.dtype)
            out_names.append(name)
            out_avals.append(jax.core.ShapedArray(shape, dtype))
            zero_outs.append(np.zeros(shape, dtype))
    n_params = len(in_names)
    n_outs = len(out_avals)
    in_names_all = in_names + out_names + ([partition_name] if partition_name else [])
    donate = tuple(range(n_params, n_params + n_outs))

    def _body(*args):
        operands = list(args)
        if partition_name is not None:
            operands.append(_b2j.partition_id_tensor())
        outs = _b2j._bass_exec_p.bind(
            *operands,
            out_avals=tuple(out_avals),
            in_names=tuple(in_names_all),
            out_names=tuple(out_names),
            lowering_input_output_aliases=(),
            sim_require_finite=True,
            sim_require_nnan=True,
            nc=nc,
        )
        return tuple(outs)

    import numpy as _np

    devices = jax.devices()[:n_cores]
    assert len(devices) == n_cores
    mesh = Mesh(_np.asarray(devices), ("core",))
    sharded = jax.jit(
        shard_map(
            _body,
            mesh=mesh,
            in_specs=(PartitionSpec("core"),) * (n_params + n_outs),
            out_specs=(PartitionSpec("core"),) * n_outs,
            check_rep=False,
        ),
        donate_argnums=donate,
        keep_unused=True,
    )
    return sharded, in_names, out_names, out_avals, zero_outs, n_params


def _cached_run_via_pjrt(nc, in_maps, n_cores):
    if nc.dbg_addr is not None or n_cores == 1:
        return _ORIG_RUN_VIA_PJRT(nc, in_maps, n_cores=n_cores)
    key = (id(nc), n_cores)
    entry = _SPMD_CACHE.get(key)
    if entry is None:
        entry = _build_spmd_entry(nc, n_cores)
        _SPMD_CACHE[key] = entry
    sharded, in_names, out_names, out_avals, zero_outs, n_params = entry
    per_core = [[np.asarray(m[nm]) for nm in in_names] for m in in_maps]
    concat_in = [
        np.concatenate([per_core[c][i] for c in range(n_cores)], axis=0)
        for i in range(n_params)
    ]
    concat_zeros = [
        np.zeros((n_cores * z.shape[0], *z.shape[1:]), z.dtype) for z in zero_outs
    ]
    out_arrs = sharded(*concat_in, *concat_zeros)
    return [
        {
            name: np.asarray(out_arrs[i]).reshape(n_cores, *out_avals[i].shape)[c]
            for i, name in enumerate(out_names)
        }
        for c in range(n_cores)
    ]


def _patched_run_via_pjrt(nc, in_maps, n_cores):
    try:
        return _cached_run_via_pjrt(nc, in_maps, n_cores)
    except Exception:
        return _ORIG_RUN_VIA_PJRT(nc, in_maps, n_cores=n_cores)


_b2j.run_bass_via_pjrt = _patched_run_via_pjrt


def build_nc(Tc):
    """Per-core kernel; inputs are mask-compacted to Tc kept positions per row.

    Layout: k arrives host-transposed as [D, BLOC, Tc] (partition = d), v as
    [Tc, BLOC, D] (partition = t), both bf16. Layer 1 uses the per-batch
    effective weight wke = Wk + diag(q_b) Wd so the whole emb concat collapses
    into one K=64 matmul per batch; the query path enters through the
    precomputed bias C = Wq.T q + b1. |Wo| is folded into W2; sign(Wo) is the
    moving operand of the score matmul.
    """
    # score/attn t-chunks (partition dim of score tiles is <= 128)
    CH = [(0, Tc)] if Tc <= 128 else [(0, 128), (128, Tc)]

    # inputs are packed into one DRAM param per dtype: each extra jit input
    # array costs ~35-85ms of axon transfer overhead per call
    nKV = D * BLOC * Tc + Tc * BLOC * D
    nPF = Tc * BLOC + BLOC * Tc + H1 + H2 + 128 * 128
    nPB = D * BLOC + 4 * D * H1 + H2
    nc = bacc.Bacc()
    kv_e = nc.declare_dram_parameter("kv", [nKV], I8, isOutput=False)
    pf_e = nc.declare_dram_parameter("pf", [nPF], F32, isOutput=False)
    pb_e = nc.declare_dram_parameter("pb", [nPB], BF16, isOutput=False)
    out_e = nc.declare_dram_parameter("out", [BLOC, D], F32, isOutput=True)

    o = 0
    kc_e = kv_e[o : o + D * BLOC * Tc].rearrange("(d b t) -> d b t", d=D, b=BLOC, t=Tc)
    o += D * BLOC * Tc
    vt_e = kv_e[o : o + Tc * BLOC * D].rearrange("(t b d) -> t b d", t=Tc, b=BLOC, d=D)

    o = 0
    rs_e = pf_e[o : o + Tc * BLOC].rearrange("(t b) -> t b", t=Tc, b=BLOC); o += Tc * BLOC
    m_e = pf_e[o : o + BLOC * Tc].rearrange("(b t) -> b t", b=BLOC, t=Tc); o += BLOC * Tc
    b1_e = pf_e[o : o + H1].rearrange("(h o) -> h o", h=H1, o=1); o += H1
    b2_e = pf_e[o : o + H2].rearrange("(h o) -> h o", h=H2, o=1); o += H2
    id_e = pf_e[o : o + 128 * 128].rearrange("(a b) -> a b", a=128, b=128)

    o = 0
    qT_e = pb_e[o : o + D * BLOC].rearrange("(d b) -> d b", d=D, b=BLOC); o += D * BLOC
    Wk_e = pb_e[o : o + D * H1].rearrange("(d h) -> d h", d=D, h=H1); o += D * H1
    Wd_e = pb_e[o : o + D * H1].rearrange("(d h) -> d h", d=D, h=H1); o += D * H1
    Wq_e = pb_e[o : o + D * H1].rearrange("(d h) -> d h", d=D, h=H1); o += D * H1
    W2_e = pb_e[o : o + H1 * H2].rearrange("(h k) -> h k", h=H1, k=H2); o += H1 * H2
    sv_e = pb_e[o : o + H2].rearrange("(h o) -> h o", h=H2, o=1)

    with TileContext(nc) as tc:
        with tc.tile_pool(name="const", bufs=1) as cp:
            Wk_s = cp.tile([D, H1], BF16, tag="Wk")
            nc.sync.dma_start(out=Wk_s[:, :], in_=Wk_e[:, :])
            Wd_s = cp.tile([D, H1], BF16, tag="Wd")
            nc.sync.dma_start(out=Wd_s[:, :], in_=Wd_e[:, :])
            Wq_s = cp.tile([D, H1], BF16, tag="Wq")
            nc.sync.dma_start(out=Wq_s[:, :], in_=Wq_e[:, :])
            W2_s = cp.tile([H1, H2], BF16, tag="W2")
            nc.sync.dma_start(out=W2_s[:, :], in_=W2_e[:, :])
            sv_s = cp.tile([H2, 1], BF16, tag="sv")
            nc.sync.dma_start(out=sv_s[:, :], in_=sv_e[:, :])
            b1_s = cp.tile([H1, 1], F32, tag="b1")
            nc.sync.dma_start(out=b1_s[:, :], in_=b1_e[:, :])
            b2_s = cp.tile([H2, 1], F32, tag="b2")
            nc.sync.dma_start(out=b2_s[:, :], in_=b2_e[:, :])
            qT_s = cp.tile([D, BLOC], BF16, tag="qT")
            nc.sync.dma_start(out=qT_s[:, :], in_=qT_e[:, :])
            id_s = cp.tile([128, 128], F32, tag="ident")
            nc.sync.dma_start(out=id_s[:, :], in_=id_e[:, :])
            C_s = cp.tile([H1, BLOC], F32, tag="C")

            # C = Wq.T @ qT + b1 : per-batch layer-1 bias (query path)
            with tc.tile_pool(name="pC", bufs=1, space="PSUM") as pCp:
                C_ps = pCp.tile([H1, BLOC], F32, tag="Cp")
                nc.tensor.matmul(C_ps[:, :], Wq_s[:, :], qT_s[:, :], start=True, stop=True)
                nc.vector.tensor_scalar_add(C_s[:, :], C_ps[:, :], b1_s[:, 0:1])

            with (
                tc.tile_pool(name="kpool", bufs=2) as kp,
                tc.tile_pool(name="vpool", bufs=2) as vp,
                tc.tile_pool(name="wpool", bufs=3) as wp,
                tc.tile_pool(name="hpool", bufs=3) as hp,
                tc.tile_pool(name="bpool", bufs=2) as bp,
                tc.tile_pool(name="spool", bufs=2) as sp,
                tc.tile_pool(name="ps_h1", bufs=2, space="PSUM") as ph1,
                tc.tile_pool(name="ps_h2", bufs=2, space="PSUM") as ph2,
                tc.tile_pool(name="ps_acc", bufs=2, space="PSUM") as pac,
                tc.tile_pool(name="ps_nat", bufs=1, space="PSUM") as pna,
                tc.tile_pool(name="ps_ot", bufs=1, space="PSUM") as pot,
            ):
                for blk in range(NBLK):
                    b0 = blk * BB
                    kT8 = kp.tile([D, BB, Tc], I8, tag="kT8")
                    nc.sync.dma_start(out=kT8[:, :, :], in_=kc_e[:, b0 : b0 + BB, :])
                    kT = kp.tile([D, BB, Tc], BF16, tag="kT")
                    nc.vector.tensor_copy(kT[:, :, :], kT8[:, :, :])  # int8 -> bf16 (exact)
                    # v chunks [t, b, d], int8 -> bf16; per-row dequant scale is
                    # applied to the attention weights (rS) instead of to v
                    vts = []
                    rss = []
                    for ci, (t0, t1) in enumerate(CH):
                        vv8 = vp.tile([t1 - t0, BB, D], I8, tag=f"v8{ci}", name=f"vv8{ci}")
                        nc.sync.dma_start(out=vv8[:, :, :], in_=vt_e[t0:t1, b0 : b0 + BB, :])
                        vv = vp.tile([t1 - t0, BB, D], BF16, tag=f"v{ci}", name=f"vv{ci}")
                        nc.vector.tensor_copy(vv[:, :, :], vv8[:, :, :])
                        vts.append(vv)
                        rs = sp.tile([t1 - t0, BB], F32, tag=f"rs{ci}", name=f"rs{ci}")
                        nc.sync.dma_start(out=rs[:, :], in_=rs_e[t0:t1, b0 : b0 + BB])
                        rss.append(rs)
                    mask_s = sp.tile([BB, Tc], F32, tag="mask")
                    nc.sync.dma_start(out=mask_s[:, :], in_=m_e[b0 : b0 + BB, :])

                    scs = [
                        pac.tile([t1 - t0, BB], F32, tag=f"acc{ci}", name=f"scT{ci}")
                        for ci, (t0, t1) in enumerate(CH)
                    ]
                    for j in range(BB):
                        b = b0 + j
                        # effective layer-1 weight for the k path:
                        # wke = Wk + diag(q_b) @ Wd
                        wke = wp.tile([D, H1], BF16, tag="wke")
                        nc.vector.scalar_tensor_tensor(
                            wke[:, :], Wd_s[:, :], qT_s[:, b : b + 1], Wk_s[:, :],
                            op0=ALU.mult, op1=ALU.add,
                        )
                        h1p = ph1.tile([H1, Tc], F32, tag="h1")
                        nc.tensor.matmul(
                            h1p[:, :], wke[:, :], kT[:, j, :], start=True, stop=True
                        )
                        h1s = hp.tile([H1, Tc], BF16, tag="h1s")
                        if j % 2 == 0:
                            nc.vector.tensor_scalar(
                                h1s[:, :], h1p[:, :], C_s[:, b : b + 1], 0.0,
                                op0=ALU.add, op1=ALU.max,
                            )
                        else:
                            nc.scalar.activation(
                                h1s[:, :], h1p[:, :], AF.Relu,
                                bias=C_s[:, b : b + 1], scale=1.0,
                            )
                        h2p = ph2.tile([H2, Tc], F32, tag="h2")
                        nc.tensor.matmul(h2p[:, :], W2_s[:, :], h1s[:, :], start=True, stop=True)
                        # z = relu(h2 + b2'), |Wo| folded into W2'; sign(Wo)
                        # applied by the score matmul below
                        zs = hp.tile([H2, Tc], BF16, tag="zs")
                        nc.scalar.activation(
                            zs[:, :], h2p[:, :], AF.Relu, bias=b2_s[:, 0:1], scale=1.0
                        )
                        for ci, (t0, t1) in enumerate(CH):
                            nc.tensor.matmul(
                                scs[ci][:, j : j + 1], zs[:, t0:t1], sv_s[:, :],
                                start=True, stop=True,
                            )

                    # ---- batched softmax over the block ----
                    scp = pna.tile([BB, Tc], F32, tag="nat")  # [b, t]
                    for ci, (t0, t1) in enumerate(CH):
                        sc_s = bp.tile([t1 - t0, BB], F32, tag=f"scs{ci}", name=f"scs{ci}")
                        nc.vector.tensor_copy(sc_s[:, :], scs[ci][:, :])
                        nc.tensor.transpose(
                            scp[:, t0:t1], sc_s[:, :], id_s[0 : t1 - t0, 0 : t1 - t0]
                        )
                    M_s = bp.tile([BB, 1], F32, tag="M")
                    nc.vector.tensor_reduce(M_s[:, :], scp[:, :], axis=AX.X, op=ALU.max)
                    negM = bp.tile([BB, 1], F32, tag="negM")
                    nc.vector.tensor_scalar_mul(negM[:, :], M_s[:, :], -1.0)
                    p_s = sp.tile([BB, Tc], F32, tag="p")
                    nc.scalar.activation(
                        p_s[:, :], scp[:, :], AF.Exp, bias=negM[:, 0:1], scale=1.0
                    )
                    pm_s = sp.tile([BB, Tc], F32, tag="pm")
                    Z_s = bp.tile([BB, 1], F32, tag="Z")
                    nc.vector.scalar_tensor_tensor(
                        pm_s[:, :], p_s[:, :], 1.0, mask_s[:, :],
                        op0=ALU.mult, op1=ALU.mult, accum_out=Z_s[:, 0:1],
                    )
                    rZ = bp.tile([BB, 1], F32, tag="rZ")
                    nc.vector.reciprocal(rZ[:, :], Z_s[:, :])

                    # transpose attn back to [t, b] columns, cast bf16
                    pTs = []
                    for ci, (t0, t1) in enumerate(CH):
                        pT_p = pac.tile([t1 - t0, BB], F32, tag=f"acc{ci}", name=f"pTp{ci}")
                        nc.tensor.transpose(
                            pT_p[:, :], pm_s[:, t0:t1], id_s[0:BB, 0:BB]
                        )
                        pT_s = bp.tile([t1 - t0, BB], BF16, tag=f"pTs{ci}", name=f"pTs{ci}")
                        nc.vector.tensor_mul(pT_s[:, :], pT_p[:, :], rss[ci][:, :])
                        pTs.append(pT_s)

                    # ---- attn @ v, output as columns [d, j] ----
                    oT = pot.tile([D, BB], F32, tag="outT")
                    for j in range(BB):
                        for ci in range(len(CH)):
                            nc.tensor.matmul(
                                oT[:, j : j + 1], vts[ci][:, j, :], pTs[ci][:, j : j + 1],
                                start=(ci == 0), stop=(ci == len(CH) - 1),
                            )
                    oT_s = bp.tile([D, BB], F32, tag="oTs")
                    nc.vector.tensor_copy(oT_s[:, :], oT[:, :])
                    oN = pna.tile([BB, D], F32, tag="nat")
                    nc.tensor.transpose(oN[:, :], oT_s[:, :], id_s[0:D, 0:D])
                    out_s = sp.tile([BB, D], F32, tag="outs")
                    nc.vector.tensor_scalar_mul(out_s[:, :], oN[:, :], rZ[:, 0:1])
                    nc.sync.dma_start(out=out_e[b0 : b0 + BB, :], in_=out_s[:, :])
    nc.compile()
    return nc


_PREP_JIT = {}


def _prep_jax(key_a, value_a, maskf, query_a, idx, Tc):
    """Fast host preprocessing on the XLA CPU backend (multithreaded)."""
    import jax
    import jax.numpy as jnp

    cpu = jax.devices("cpu")[0]

    if Tc not in _PREP_JIT:

        def f(k, v, mf, q, ix):
            kc = jnp.take_along_axis(k, ix[:, :, None], axis=1)
            kq = jnp.clip(jnp.round(kc * KSCALE), -127.0, 127.0).astype(jnp.int8)
            kc8 = kq.reshape(NCORES, BLOC, Tc, D).transpose(0, 3, 1, 2)
            vc = jnp.take_along_axis(v, ix[:, :, None], axis=1)
            amax = jnp.maximum(jnp.abs(vc).max(axis=2, keepdims=True), 1e-20)
            vsc = 127.0 / amax
            v8 = jnp.round(vc * vsc).astype(jnp.int8)
            vt8 = v8.reshape(NCORES, BLOC, Tc, D).transpose(0, 2, 1, 3)
            rs8 = (1.0 / vsc)[:, :, 0].reshape(NCORES, BLOC, Tc).transpose(0, 2, 1)
            mc8 = jnp.take_along_axis(mf, ix, axis=1).reshape(NCORES, BLOC, Tc)
            q8 = q.reshape(NCORES, BLOC, D).transpose(0, 2, 1).astype(jnp.bfloat16)
            kv8 = jnp.concatenate(
                [kc8.reshape(NCORES, -1), vt8.reshape(NCORES, -1)], axis=1
            )
            pf8 = jnp.concatenate(
                [rs8.reshape(NCORES, -1), mc8.reshape(NCORES, -1)], axis=1
            )
            return kv8, pf8, q8

        _PREP_JIT[Tc] = jax.jit(f)

    with jax.default_device(cpu):
        outs = _PREP_JIT[Tc](key_a, value_a, maskf, query_a, idx)
        return tuple(np.asarray(o) for o in outs)


def _prep_np(key_a, value_a, maskf, query_a, idx, Tc):
    kc = np.take_along_axis(key_a, idx[:, :, None], axis=1)
    kq = np.clip(np.round(kc * KSCALE), -127.0, 127.0).astype(np.int8)
    kc8 = np.ascontiguousarray(kq.reshape(NCORES, BLOC, Tc, D).transpose(0, 3, 1, 2))
    vc = np.take_along_axis(value_a, idx[:, :, None], axis=1)
    amax = np.maximum(np.abs(vc).max(axis=2, keepdims=True), 1e-20)
    vsc = 127.0 / amax
    v8 = np.round(vc * vsc).astype(np.int8)
    vt8 = np.ascontiguousarray(v8.reshape(NCORES, BLOC, Tc, D).transpose(0, 2, 1, 3))
    rs8 = np.ascontiguousarray(
        (1.0 / vsc)[:, :, 0].reshape(NCORES, BLOC, Tc).transpose(0, 2, 1)
    )
    mc8 = np.take_along_axis(maskf, idx, axis=1).reshape(NCORES, BLOC, Tc)
    q8 = np.ascontiguousarray(
        query_a.reshape(NCORES, BLOC, D).transpose(0, 2, 1).astype(NPBF)
    )
    kv8 = np.concatenate([kc8.reshape(NCORES, -1), vt8.reshape(NCORES, -1)], axis=1)
    pf8 = np.concatenate([rs8.reshape(NCORES, -1), mc8.reshape(NCORES, -1)], axis=1)
    return kv8, pf8, q8


def kernel(query, key, value, mask, W1, b1, W2, b2, Wo, bo, **kw):
    query = np.ascontiguousarray(np.asarray(query, dtype=np.float32))
    key = np.ascontiguousarray(np.asarray(key, dtype=np.float32))
    value = np.ascontiguousarray(np.asarray(value, dtype=np.float32))
    mask_i = np.asarray(mask)
    W1 = np.asarray(W1, dtype=np.float32)
    b1v = np.asarray(b1, dtype=np.float32).reshape(H1, 1)
    W2a = np.asarray(W2, dtype=np.float32)
    b2v = np.asarray(b2, dtype=np.float32).reshape(H2)
    Wov = np.asarray(Wo, dtype=np.float32).reshape(H2)
    # bo shifts all scores equally -> softmax-invariant; dropped.

    # --- mask compaction: keep only unmasked t positions (padded with rows
    # whose mask is 0), so k/v transfer bytes scale with max kept count ---
    keep = mask_i != 0
    cnt = keep.sum(axis=1)
    maxc = int(cnt.max()) if cnt.size else 0
    Tc = min(T, max(2, (maxc + 1) // 2 * 2))
    order = np.argsort(~keep, axis=1, kind="stable").astype(np.int32)
    idx = np.ascontiguousarray(order[:, :Tc])
    maskf = keep.astype(np.float32)

    try:
        kv8, pf8, q8 = _prep_jax(key, value, maskf, query, idx, Tc)
    except Exception:
        kv8, pf8, q8 = _prep_np(key, value, maskf, query, idx, Tc)

    # weight transforms: emb = [q, k, q-k, q*k] @ [W1a; W1b; W1c; W1d]
    #   k path:   Wk = W1b - W1c
    #   q path:   Wq = W1a + W1c  (-> C bias, computed on device)
    #   q*k path: Wd = W1d (applied via wke = Wk + diag(q) Wd)
    W1a, W1b, W1c, W1d = W1[0:D], W1[D : 2 * D], W1[2 * D : 3 * D], W1[3 * D : 4 * D]
    # 1/KSCALE (k int8 dequant) is folded into the k-path weights
    Wk = np.ascontiguousarray((W1b - W1c) / KSCALE).astype(NPBF)
    Wq = np.ascontiguousarray(W1a + W1c).astype(NPBF)
    Wd = np.ascontiguousarray(W1d / KSCALE).astype(NPBF)
    aWo = np.abs(Wov)
    W2p = np.ascontiguousarray(W2a * aWo[None, :]).astype(NPBF)
    b2p = np.ascontiguousarray((b2v * aWo).reshape(H2, 1)).astype(np.float32)
    svec = np.sign(Wov).reshape(H2, 1).astype(NPBF)
    ident = np.eye(128, dtype=np.float32)
    constf = np.concatenate(
        [b1v.ravel(), b2p.ravel(), ident.ravel()]
    ).astype(np.float32)
    constb = np.concatenate(
        [Wk.ravel(), Wd.ravel(), Wq.ravel(), W2p.ravel(), svec.ravel()]
    ).astype(NPBF)

    if Tc not in _cached:
        _cached[Tc] = build_nc(Tc)
    nc = _cached[Tc]

    in_maps = []
    for c in range(NCORES):
        in_maps.append(
            {
                "kv": kv8[c],
                "pf": np.concatenate([pf8[c], constf]),
                "pb": np.concatenate([q8[c].ravel(), constb]),
            }
        )
    global _last_in_maps, _last_nc
    _last_in_maps = in_maps
    _last_nc = nc
    res = run_bass_kernel_spmd(nc, in_maps, list(range(NCORES)))
    outs = res.results if hasattr(res, "results") else res
    full = np.empty((B, D), dtype=np.float32)
    for c in range(NCORES):
        r = outs[c]
        arr = r["out"] if isinstance(r, dict) else r
        full[c * BLOC : (c + 1) * BLOC] = np.asarray(arr).reshape(BLOC, D)
    return full
